# revision 21
# baseline (speedup 1.0000x reference)
"""Trainium2 Bass kernel for triple-head Bahdanau attention (nn_Attention_48258252537865).

v5: like v4 (fp8 DoubleRow u-matmuls + host-side quantization-error
compensation) but the three heads' score matmuls are packed into distinct
PE column groups (tile_position col packing): head t's [K=128, M=1] score
matmul writes PSUM partition 32*t, so the three matmuls execute concurrently
in different 32-column strips of the array instead of serializing three
512-cycle streams on column group 0.  The loop is restructured j-outer /
t-inner so each j's three score matmuls are emitted back-to-back.
"""

import numpy as np
import ml_dtypes
from contextlib import ExitStack

S = 8192
H2 = 1024
A = 2048
NCORES = 8
NEG = -1.0e30
SX = 2.0 ** 5          # sentence pre-scale (fp8)
SW = 2.0 ** 9          # weight pre-scale (fp8)
SCALE_INV = 1.0 / (SX * SW)
NP_F8 = ml_dtypes.float8_e4m3
NP_BF16 = ml_dtypes.bfloat16

_cache = {}
LAST_RESULTS = None  # BassKernelResults of the most recent device run


def _build(S_local):
    import concourse.bacc as bacc
    import concourse.tile as tile
    from concourse import mybir

    F32 = mybir.dt.float32
    F32R = mybir.dt.float32r
    F8 = mybir.dt.float8e4
    BF16 = mybir.dt.bfloat16
    TANH = mybir.ActivationFunctionType.Tanh
    EXP = mybir.ActivationFunctionType.Exp
    DR = mybir.MatmulPerfMode.DoubleRow

    KT = H2 // 128                      # 8 fp8 k-slices of the contraction
    KT2 = KT // 2                       # 4 DoubleRow K=256 tiles
    NJ = A // 128                       # a-tiles per head
    ST = S_local // 128                 # s-tiles (transpose/numerator)
    SC = [(c, min(512, S_local - c)) for c in range(0, S_local, 512)]

    nc = bacc.Bacc("TRN2", target_bir_lowering=False, debug=False,
                   num_devices=NCORES)

    sentT8_d = nc.dram_tensor("sentT8", [128, KT * S_local], F8,
                              kind="ExternalInput")
    sentbf_d = nc.dram_tensor("sentbf", [128, ST * H2], BF16,
                              kind="ExternalInput")
    Wt8_d = nc.dram_tensor("Wt8", [3, NJ, 128, KT * 128], F8,
                           kind="ExternalInput")
    Vt_d = nc.dram_tensor("Vt", [128, NJ * 3], BF16, kind="ExternalInput")
    Bt_d = nc.dram_tensor("Bt", [128, 3 * NJ], F32, kind="ExternalInput")
    corr3_d = nc.dram_tensor("corr3", [3, S_local], F32R, kind="ExternalInput")
    sel67_d = nc.dram_tensor("sel67", [3, 67], F32R, kind="ExternalInput")
    id67_d = nc.dram_tensor("id67", [67, 67], F32, kind="ExternalInput")

    Ncore_d = nc.dram_tensor("Ncore", [3, H2], F32, kind="ExternalOutput")
    z4_d = nc.dram_tensor("z4", [3, 4], F32, kind="ExternalOutput")

    with tile.TileContext(nc) as tc, ExitStack() as ctx:
        const = ctx.enter_context(tc.tile_pool(name="const", bufs=1))
        wpool = ctx.enter_context(tc.tile_pool(name="w", bufs=5))
        thpool = ctx.enter_context(tc.tile_pool(name="th", bufs=7))
        ph1 = ExitStack()
        upool = ph1.enter_context(tc.tile_pool(name="u", bufs=3, space="PSUM"))
        epool = ph1.enter_context(tc.tile_pool(name="e", bufs=1, space="PSUM"))

        # ---- startup (see v4 notes): critical deps first per ring ----
        Wt_sb = {}

        def _wdma(t, j):
            w = wpool.tile([128, KT, 128], F8, tag="w")
            nc.sync.dma_start(
                w[:].rearrange("p k a -> p (k a)"), Wt8_d.ap()[t, j])
            Wt_sb[(t, j)] = w

        sel67_sb = const.tile([3, 67], F32R, tag="sel67")
        id67_sb = const.tile([67, 67], F32, tag="id67")
        Vt_sb = const.tile([128, NJ * 3], BF16, tag="vt")
        Bt_sb = const.tile([128, 3 * NJ], F32, tag="bt")
        corr_sb = const.tile([3, S_local], F32R, tag="corr")
        KH = KT // 2
        nc.gpsimd.dma_start(sel67_sb[:], sel67_d.ap()[:])
        nc.gpsimd.dma_start(Bt_sb[:], Bt_d.ap()[:])
        nc.gpsimd.dma_start(Vt_sb[:], Vt_d.ap()[:])
        nc.gpsimd.dma_start(corr_sb[:], corr3_d.ap()[:])
        nc.gpsimd.dma_start(id67_sb[:], id67_d.ap()[:])
        _wdma(0, 0)
        sentT_cs = []
        for ci, (c, n) in enumerate(SC):
            sct = const.tile([128, KT, n], F8, tag=f"sentT{ci}", name=f"sT{ci}")
            base = ci * KT * 512
            nc.scalar.dma_start(
                sct[:, 0:KH, :].rearrange("p k s -> p (k s)"),
                sentT8_d.ap()[:, base: base + KH * n])
            sentT_cs.append(sct)
        for ci, (c, n) in enumerate(SC):
            base = ci * KT * 512
            nc.gpsimd.dma_start(
                sentT_cs[ci][:, KH:KT, :].rearrange("p k s -> p (k s)"),
                sentT8_d.ap()[:, base + KH * n: base + KT * n])
        _wdma(1, 0)
        _wdma(2, 0)
        _wdma(0, 1)

        sent_sb = const.tile([128, ST * H2], BF16, tag="sent")

        # ---- score accumulator: head t's row on PSUM partition 32t ----
        e9_ps = epool.tile([67, S_local], F32, tag="e")

        # ---- PE pre-warm: ~3us of tiny matmuls fed by a memset tile (no
        # DMA dep) keep the HAM activity window busy during startup DMA so
        # the first real matmuls run at 2.4GHz instead of 1.2 ----
        dm_sb = const.tile([3, 512], F32, tag="dm")
        nc.vector.memset(dm_sb[:], 1.0)
        dmr = dm_sb[:].bitcast(F32R)
        for _ in range(10):
            nc.tensor.matmul(e9_ps[0:3, 0:512], dmr[:, 0:3], dmr,
                             start=True, stop=True)

        # ---- u (fp8 DoubleRow) -> tanh -> col-packed score triples ----
        pend = None  # [th0, th1, th2] of the previous j
        for j in range(NJ):
            ths = []
            for t in range(3):
                wtile = Wt_sb.pop((t, j), None)
                if wtile is None:
                    wtile = wpool.tile([128, KT, 128], F8, tag="w")
                    nc.sync.dma_start(
                        wtile[:].rearrange("p k a -> p (k a)"), Wt8_d.ap()[t, j])
                u_ps = upool.tile([128, S_local], F32, tag="u")
                if j == 0 and t == 0:
                    for ci, (c, n) in enumerate(SC):
                        for kt in range(KT2):
                            nc.tensor.matmul(
                                u_ps[:, c:c + n],
                                wtile[:, 2 * kt:2 * kt + 2, :],
                                sentT_cs[ci][:, 2 * kt:2 * kt + 2, :],
                                start=(kt == 0), stop=(kt == KT2 - 1),
                                perf_mode=DR)
                else:
                    for kt in range(KT2):
                        for ci, (c, n) in enumerate(SC):
                            nc.tensor.matmul(
                                u_ps[:, c:c + n],
                                wtile[:, 2 * kt:2 * kt + 2, :],
                                sentT_cs[ci][:, 2 * kt:2 * kt + 2, :],
                                start=(kt == 0), stop=(kt == KT2 - 1),
                                perf_mode=DR)
                if t == 1 and pend is not None:
                    # score triples of j-1: per chunk, the 3 heads go to
                    # column groups 0/32/64 and run concurrently
                    for (c, n) in SC:
                        for pt in range(3):
                            nc.tensor.matmul(
                                e9_ps[32 * pt:32 * pt + 1, c:c + n],
                                Vt_sb[:, 3 * (j - 1) + pt: 3 * (j - 1) + pt + 1],
                                pend[pt][:, c:c + n],
                                start=False, stop=False)
                th = thpool.tile([128, S_local], BF16, tag="th",
                                 name=f"th{t}_{j}")
                if j == NJ - 1:
                    # final j: per-chunk tanh so the last score triples can
                    # start as soon as their chunk is ready
                    for (c, n) in SC:
                        nc.scalar.activation(
                            th[:, c:c + n], u_ps[:, c:c + n], TANH,
                            bias=Bt_sb[:, j * 3 + t: j * 3 + t + 1],
                            scale=SCALE_INV)
                else:
                    nc.scalar.activation(th[:], u_ps[:], TANH,
                                         bias=Bt_sb[:, j * 3 + t: j * 3 + t + 1],
                                         scale=SCALE_INV)
                ths.append(th)
                if 1 <= j <= 3 and (j - 1) * 3 + t < ST:
                    # trickle the 2MB numerator operand behind the W stream
                    pc = (j - 1) * 3 + t
                    nc.sync.dma_start(sent_sb[:, pc * H2:(pc + 1) * H2],
                                      sentbf_d.ap()[:, pc * H2:(pc + 1) * H2])
                if j == 0 and t == 0:
                    # key mask + fp8 error compensation: one K=3 matmul with
                    # a [3, 67] selection stationary scatters head t's corr
                    # row onto partition 32t and zero-starts the accumulator
                    for (c, n) in SC:
                        nc.tensor.matmul(
                            e9_ps[0:67, c:c + n], sel67_sb[:],
                            corr_sb[0:3, c:c + n],
                            start=True, stop=False)
            pend = ths
        for (c, n) in SC:
            for pt in range(3):
                nc.tensor.matmul(
                    e9_ps[32 * pt:32 * pt + 1, c:c + n],
                    Vt_sb[:, 3 * (NJ - 1) + pt: 3 * (NJ - 1) + pt + 1],
                    pend[pt][:, c:c + n],
                    start=False, stop=True)

        # ---- exp over all 67 partitions at once (rows other than
        # 0/32/64 hold exactly 0.0 from the start=True scatter, so they
        # exp to 1.0 and are simply never read downstream) ----
        e9x_sb = const.tile([67, S_local], F32, tag="e9x")
        SCE = []
        _c = 0
        while _c < S_local:
            _n = min(128 if _c < 256 else (256 if _c < 512 else 512),
                     S_local - _c)
            SCE.append((_c, _n))
            _c += _n
        z9_sb = const.tile([67, len(SCE)], F32, tag="z9")
        z3raw = const.tile([3, len(SCE)], F32, tag="z3raw")
        rings = [nc.sync, nc.scalar, nc.gpsimd]
        for ci, (c, n) in enumerate(SCE):
            nc.scalar.activation(e9x_sb[0:67, c:c + n], e9_ps[0:67, c:c + n],
                                 EXP, accum_out=z9_sb[0:67, ci:ci + 1])
        for tt in range(3):
            rings[tt].dma_start(z3raw[tt:tt + 1, :],
                                z9_sb[32 * tt:32 * tt + 1, :])
        nc.scalar.dma_start(z4_d.ap()[:, 0:len(SCE)], z3raw[:])

        ph1.close()  # free u/e PSUM banks for the epilogue pools

        # ---- fused epilogue: transpose exp-scores per s-tile, accumulate
        # numerator N[t, :] = sum_s exp_scores[t, s] * sent[s, :] ----
        trpool = ctx.enter_context(tc.tile_pool(name="tr", bufs=3, space="PSUM"))
        npool = ctx.enter_context(tc.tile_pool(name="n", bufs=2, space="PSUM"))
        eT_sb = const.tile([128, 3 * ST], BF16, tag="eT")
        n_ps = []
        for _hi in range(H2 // 512):
            n_ps_hi = npool.tile([3, 512], F32, tag="n")
            n_ps.append(n_ps_hi)
        def _num_mm(k):
            for hi, hc in enumerate(range(0, H2, 512)):
                nc.tensor.matmul(n_ps[hi][0:3, :],
                                 eT_sb[:, 3 * k:3 * k + 3],
                                 sent_sb[:, k * H2 + hc: k * H2 + hc + 512],
                                 start=(k == 0), stop=(k == ST - 1))
        for k in range(ST):
            tr_ps = trpool.tile([128, 67], F32, tag="tr")
            nc.tensor.transpose(tr_ps[:], e9x_sb[0:67, k * 128:(k + 1) * 128],
                                id67_sb[:])
            # gather head columns {0,32,64}; alternate engines to pipeline
            if k % 2 == 0:
                nc.vector.tensor_copy(eT_sb[:, 3 * k:3 * k + 3],
                                      tr_ps[:, 0:65:32])
            else:
                nc.scalar.copy(eT_sb[:, 3 * k:3 * k + 3], tr_ps[:, 0:65:32])
            if k >= 2:  # numerator MMs two tiles behind: copies never stall PE
                _num_mm(k - 2)
        for k in range(max(0, ST - 2), ST):
            _num_mm(k)
        n_sb = const.tile([3, H2], F32, tag="nsb")
        for hi, hc in enumerate(range(0, H2, 512)):
            if hi % 2 == 0:
                nc.vector.tensor_copy(n_sb[:, hc:hc + 512], n_ps[hi][:])
            else:
                nc.scalar.copy(n_sb[:, hc:hc + 512], n_ps[hi][:])
            nc.sync.dma_start(Ncore_d.ap()[:, hc:hc + 512], n_sb[:, hc:hc + 512])

    nc.compile()
    return nc


def kernel(**inputs):
    global LAST_RESULTS
    from concourse import bass_utils

    sentence = np.ascontiguousarray(
        np.asarray(inputs["sentence"], dtype=np.float32)[0])      # [S, H2]
    length = int(np.asarray(inputs["length"]).reshape(-1)[0])
    if length <= 0:
        return np.zeros((1, H2), dtype=np.float32)
    length = min(length, S)

    ctxs = [inputs["pos_embedding"], inputs["cardinal_phrase_embedding"],
            inputs["headline_embedding"]]
    tags = ["p", "c", "h"]

    # ---- host prep: quantize, fold biases, fp8 error compensation ----
    x8 = (sentence * SX).astype(NP_F8)                            # [S, H2] fp8
    x8f = x8.astype(np.float32) / SX
    dx = sentence - x8f

    bias_all = np.empty((3, A), dtype=np.float32)
    W8_all = np.empty((3, H2, A), dtype=NP_F8)
    v_all = np.empty((3, A), dtype=np.float32)
    corr_all = np.empty((3, S), dtype=np.float32)
    sub = np.arange(0, S, 16)                                     # c_a sample
    for i, tg in enumerate(tags):
        ctx = np.asarray(ctxs[i], dtype=np.float32)[0]            # [E]
        bias = (np.asarray(inputs[f"b_sent_{tg}"], dtype=np.float32)
                + ctx @ np.asarray(inputs[f"W_ctx_{tg}"], dtype=np.float32)
                + np.asarray(inputs[f"b_ctx_{tg}"], dtype=np.float32))
        W = np.asarray(inputs[f"W_sent_{tg}"], dtype=np.float32)
        v = np.asarray(inputs[f"v_{tg}"], dtype=np.float32)
        W8 = (W * SW).astype(NP_F8)
        W8f = W8.astype(np.float32) / SW
        dW = W - W8f
        u_sub = x8f[sub] @ W8f + bias
        c_a = (1.0 - np.tanh(u_sub) ** 2).mean(axis=0)            # [A]
        vc = v * c_a
        corr_all[i] = dx @ (W @ vc) + x8f @ (dW @ vc)
        bias_all[i] = bias
        W8_all[i] = W8
        v_all[i] = v

    S_local = max(128, -(-length // (NCORES * 128)) * 128)        # ceil, 128-aligned
    nc = _cache.get(S_local)
    if nc is None:
        nc = _build(S_local)
        _cache[S_local] = nc

    NJ = A // 128
    KT = H2 // 128
    # Wt8[t, j][p, kt*128 + a] = W8[t, kt*128 + p, j*128 + a]
    Wt8 = np.ascontiguousarray(
        W8_all.reshape(3, KT, 128, NJ, 128)
              .transpose(0, 3, 2, 1, 4).reshape(3, NJ, 128, KT * 128))
    # Vt[p, j*3 + t] = v_t[j*128 + p]
    Vt = np.ascontiguousarray(
        v_all.T.reshape(NJ, 128, 3).transpose(1, 0, 2)
             .reshape(128, NJ * 3)).astype(NP_BF16)
    Bt = np.ascontiguousarray(
        bias_all.T.reshape(NJ, 128, 3).transpose(1, 0, 2).reshape(128, 3 * NJ))
    id67 = np.eye(67, dtype=np.float32)
    sel67 = np.zeros((3, 67), dtype=np.float32)
    for t in range(3):
        sel67[t, 32 * t] = 1.0

    # overflow guard for the shift-free exp: |e| <= ||v||_1 + max|corr|
    e_bound = max(float(np.abs(v_all[t]).sum() + np.abs(corr_all[t]).max())
                  for t in range(3))
    shift = max(0.0, e_bound - 60.0)   # exp arg stays < 60 -> < 1.2e26, Z safe
    if shift:
        corr_all -= shift              # common across cores: cancels in N/Z

    ST = S_local // 128
    SCE_N = 0
    _c = 0
    while _c < S_local:
        _n = min(128 if _c < 256 else (256 if _c < 512 else 512),
                 S_local - _c)
        SCE_N += 1
        _c += _n

    in_maps = []
    for c in range(NCORES):
        s0 = c * S_local
        sl8 = x8[s0:s0 + S_local]
        slf = sentence[s0:s0 + S_local]
        if sl8.shape[0] < S_local:                                 # pad tail core
            pad = S_local - sl8.shape[0]
            sl8 = np.concatenate([sl8, np.zeros((pad, H2), NP_F8)], axis=0)
            slf = np.concatenate([slf, np.zeros((pad, H2), np.float32)], axis=0)
        # chunk-major: sentT8[p, ci*KT*512 + k*n + s'] = x8[s0+ci*512+s', k*128+p]
        slT = sl8.T.reshape(KT, 128, S_local)                      # [k, p, s]
        blocks = [
            np.ascontiguousarray(slT[:, :, cc:cc + nn].transpose(1, 0, 2)
                                 .reshape(128, KT * nn))
            for cc, nn in [(cc0, min(512, S_local - cc0))
                           for cc0 in range(0, S_local, 512)]]
        sentT8 = np.ascontiguousarray(np.concatenate(blocks, axis=1))
        # sentbf[p, k*H2 + h] = sentence[s0 + k*128 + p, h]
        sentbf = np.ascontiguousarray(
            slf.reshape(ST, 128, H2).transpose(1, 0, 2)
               .reshape(128, ST * H2)).astype(NP_BF16)
        smask = np.where((s0 + np.arange(S_local))[None, :] < length,
                         0.0, NEG).astype(np.float32)
        corr3 = np.ascontiguousarray(
            corr_all[:, s0:s0 + S_local] if s0 + S_local <= S else
            np.pad(corr_all[:, s0:S], ((0, 0), (0, s0 + S_local - S))))
        corr3 = (corr3 + smask).astype(np.float32)
        in_maps.append(dict(
            sentT8=sentT8, sentbf=sentbf, Wt8=Wt8, Vt=Vt, Bt=Bt,
            corr3=corr3, sel67=sel67, id67=id67,
        ))

    res = bass_utils.run_bass_kernel_spmd(nc, in_maps,
                                          core_ids=list(range(NCORES)))
    LAST_RESULTS = res

    # ---- exact cross-core combine (shared exp shift cancels in N/Z) ----
    Z = np.stack([res.results[c]["z4"] for c in range(NCORES)])    # [8,3,4]
    Ncore = np.stack([res.results[c]["Ncore"] for c in range(NCORES)])
    Zt = Z[:, :, :SCE_N].astype(np.float64).sum(axis=(0, 2))       # [3]
    Nt = Ncore.astype(np.float64).sum(axis=0)                      # [3,H2]
    out = (Nt / Zt[:, None]).mean(axis=0)
    return out[None, :].astype(np.float32)


# revision 23
# speedup vs baseline: 1.0151x; 1.0151x over previous
"""Trainium2 Bass kernel for triple-head Bahdanau attention (nn_Attention_48258252537865).

v5: like v4 (fp8 DoubleRow u-matmuls + host-side quantization-error
compensation) but the three heads' score matmuls are packed into distinct
PE column groups (tile_position col packing): head t's [K=128, M=1] score
matmul writes PSUM partition 32*t, so the three matmuls execute concurrently
in different 32-column strips of the array instead of serializing three
512-cycle streams on column group 0.  The loop is restructured j-outer /
t-inner so each j's three score matmuls are emitted back-to-back.
"""

import numpy as np
import ml_dtypes
from contextlib import ExitStack

S = 8192
H2 = 1024
A = 2048
NCORES = 8
NEG = -1.0e30
SX = 2.0 ** 5          # sentence pre-scale (fp8)
SW = 2.0 ** 9          # weight pre-scale (fp8)
SCALE_INV = 1.0 / (SX * SW)
NP_F8 = ml_dtypes.float8_e4m3
NP_BF16 = ml_dtypes.bfloat16

_cache = {}
LAST_RESULTS = None  # BassKernelResults of the most recent device run


def _build(S_local):
    import concourse.bacc as bacc
    import concourse.tile as tile
    from concourse import mybir

    F32 = mybir.dt.float32
    F32R = mybir.dt.float32r
    F8 = mybir.dt.float8e4
    BF16 = mybir.dt.bfloat16
    TANH = mybir.ActivationFunctionType.Tanh
    EXP = mybir.ActivationFunctionType.Exp
    DR = mybir.MatmulPerfMode.DoubleRow

    KT = H2 // 128                      # 8 fp8 k-slices of the contraction
    KT2 = KT // 2                       # 4 DoubleRow K=256 tiles
    NJ = A // 128                       # a-tiles per head
    ST = S_local // 128                 # s-tiles (transpose/numerator)
    SC = [(c, min(512, S_local - c)) for c in range(0, S_local, 512)]

    nc = bacc.Bacc("TRN2", target_bir_lowering=False, debug=False,
                   num_devices=NCORES)

    sentT8_d = nc.dram_tensor("sentT8", [128, KT * S_local], F8,
                              kind="ExternalInput")
    sentbf_d = nc.dram_tensor("sentbf", [128, ST * H2], BF16,
                              kind="ExternalInput")
    Wt8_d = nc.dram_tensor("Wt8", [3, NJ, 128, KT * 128], F8,
                           kind="ExternalInput")
    Vt_d = nc.dram_tensor("Vt", [128, NJ * 3], BF16, kind="ExternalInput")
    Bt_d = nc.dram_tensor("Bt", [128, 3 * NJ], F32, kind="ExternalInput")
    corr3_d = nc.dram_tensor("corr3", [3, S_local], F32R, kind="ExternalInput")
    sel67_d = nc.dram_tensor("sel67", [3, 67], F32R, kind="ExternalInput")
    id67_d = nc.dram_tensor("id67", [67, 67], F32, kind="ExternalInput")

    Ncore_d = nc.dram_tensor("Ncore", [3, H2], F32, kind="ExternalOutput")
    z4_d = nc.dram_tensor("z4", [3, 4], F32, kind="ExternalOutput")

    with tile.TileContext(nc) as tc, ExitStack() as ctx:
        const = ctx.enter_context(tc.tile_pool(name="const", bufs=1))
        wpool = ctx.enter_context(tc.tile_pool(name="w", bufs=5))
        thpool = ctx.enter_context(tc.tile_pool(name="th", bufs=7))
        ph1 = ExitStack()
        upool = ph1.enter_context(tc.tile_pool(name="u", bufs=3, space="PSUM"))
        epool = ph1.enter_context(tc.tile_pool(name="e", bufs=1, space="PSUM"))

        # ---- startup (see v4 notes): critical deps first per ring ----
        Wt_sb = {}

        def _wdma(t, j):
            w = wpool.tile([128, KT, 128], F8, tag="w")
            nc.sync.dma_start(
                w[:].rearrange("p k a -> p (k a)"), Wt8_d.ap()[t, j])
            Wt_sb[(t, j)] = w

        sel67_sb = const.tile([3, 67], F32R, tag="sel67")
        id67_sb = const.tile([67, 67], F32, tag="id67")
        Vt_sb = const.tile([128, NJ * 3], BF16, tag="vt")
        Bt_sb = const.tile([128, 3 * NJ], F32, tag="bt")
        corr_sb = const.tile([3, S_local], F32R, tag="corr")
        KH = KT // 2
        w00 = wpool.tile([128, KT, 128], F8, tag="w", name="w00")
        nc.sync.dma_start(w00[:, 0:KH, :].rearrange("p k a -> p (k a)"),
                          Wt8_d.ap()[0, 0][:, 0:KH * 128])
        nc.sync.dma_start(w00[:, KH:KT, :].rearrange("p k a -> p (k a)"),
                          Wt8_d.ap()[0, 0][:, KH * 128:KT * 128])
        Wt_sb[(0, 0)] = w00
        sentT_cs = []
        KQ = KH // 2
        for ci, (c, n) in enumerate(SC):
            sct = const.tile([128, KT, n], F8, tag=f"sentT{ci}", name=f"sT{ci}")
            base = ci * KT * 512
            if ci == 0:
                nc.scalar.dma_start(
                    sct[:, 0:KQ, :].rearrange("p k s -> p (k s)"),
                    sentT8_d.ap()[:, base: base + KQ * n])
                nc.scalar.dma_start(
                    sct[:, KQ:KH, :].rearrange("p k s -> p (k s)"),
                    sentT8_d.ap()[:, base + KQ * n: base + KH * n])
            else:
                nc.scalar.dma_start(
                    sct[:, 0:KH, :].rearrange("p k s -> p (k s)"),
                    sentT8_d.ap()[:, base: base + KH * n])
            sentT_cs.append(sct)
        for ci, (c, n) in enumerate(SC):
            base = ci * KT * 512
            nc.gpsimd.dma_start(
                sentT_cs[ci][:, KH:KT, :].rearrange("p k s -> p (k s)"),
                sentT8_d.ap()[:, base + KH * n: base + KT * n])
        nc.gpsimd.dma_start(Bt_sb[:], Bt_d.ap()[:])
        nc.gpsimd.dma_start(sel67_sb[:], sel67_d.ap()[:])
        nc.gpsimd.dma_start(corr_sb[:], corr3_d.ap()[:])
        nc.gpsimd.dma_start(Vt_sb[:], Vt_d.ap()[:])
        nc.gpsimd.dma_start(id67_sb[:], id67_d.ap()[:])
        _wdma(1, 0)
        _wdma(2, 0)
        _wdma(0, 1)

        sent_sb = const.tile([128, ST * H2], BF16, tag="sent")

        # ---- score accumulator: head t's row on PSUM partition 32t ----
        e9_ps = epool.tile([67, S_local], F32, tag="e")

        # ---- u (fp8 DoubleRow) -> tanh -> col-packed score triples ----
        pend = None  # [th0, th1, th2] of the previous j
        for j in range(NJ):
            ths = []
            for t in range(3):
                wtile = Wt_sb.pop((t, j), None)
                if wtile is None:
                    wtile = wpool.tile([128, KT, 128], F8, tag="w")
                    nc.sync.dma_start(
                        wtile[:].rearrange("p k a -> p (k a)"), Wt8_d.ap()[t, j])
                u_ps = upool.tile([128, S_local], F32, tag="u")
                if j == 0 and t == 0:
                    for ci, (c, n) in enumerate(SC):
                        for kt in range(KT2):
                            nc.tensor.matmul(
                                u_ps[:, c:c + n],
                                wtile[:, 2 * kt:2 * kt + 2, :],
                                sentT_cs[ci][:, 2 * kt:2 * kt + 2, :],
                                start=(kt == 0), stop=(kt == KT2 - 1),
                                perf_mode=DR)
                else:
                    for kt in range(KT2):
                        for ci, (c, n) in enumerate(SC):
                            nc.tensor.matmul(
                                u_ps[:, c:c + n],
                                wtile[:, 2 * kt:2 * kt + 2, :],
                                sentT_cs[ci][:, 2 * kt:2 * kt + 2, :],
                                start=(kt == 0), stop=(kt == KT2 - 1),
                                perf_mode=DR)
                if t == 1 and pend is not None:
                    # score triples of j-1: per chunk, the 3 heads go to
                    # column groups 0/32/64 and run concurrently
                    for (c, n) in SC:
                        for pt in range(3):
                            nc.tensor.matmul(
                                e9_ps[32 * pt:32 * pt + 1, c:c + n],
                                Vt_sb[:, 3 * (j - 1) + pt: 3 * (j - 1) + pt + 1],
                                pend[pt][:, c:c + n],
                                start=False, stop=False)
                th = thpool.tile([128, S_local], BF16, tag="th",
                                 name=f"th{t}_{j}")
                if j == NJ - 1:
                    # final j: per-chunk tanh so the last score triples can
                    # start as soon as their chunk is ready
                    for (c, n) in SC:
                        nc.scalar.activation(
                            th[:, c:c + n], u_ps[:, c:c + n], TANH,
                            bias=Bt_sb[:, j * 3 + t: j * 3 + t + 1],
                            scale=SCALE_INV)
                else:
                    nc.scalar.activation(th[:], u_ps[:], TANH,
                                         bias=Bt_sb[:, j * 3 + t: j * 3 + t + 1],
                                         scale=SCALE_INV)
                ths.append(th)
                if 1 <= j <= 3 and (j - 1) * 3 + t < ST:
                    # trickle the 2MB numerator operand behind the W stream
                    pc = (j - 1) * 3 + t
                    nc.sync.dma_start(sent_sb[:, pc * H2:(pc + 1) * H2],
                                      sentbf_d.ap()[:, pc * H2:(pc + 1) * H2])
                if j == 0 and t == 0:
                    # key mask + fp8 error compensation: one K=3 matmul with
                    # a [3, 67] selection stationary scatters head t's corr
                    # row onto partition 32t and zero-starts the accumulator
                    for (c, n) in SC:
                        nc.tensor.matmul(
                            e9_ps[0:67, c:c + n], sel67_sb[:],
                            corr_sb[0:3, c:c + n],
                            start=True, stop=False)
            pend = ths
        for (c, n) in SC:
            for pt in range(3):
                nc.tensor.matmul(
                    e9_ps[32 * pt:32 * pt + 1, c:c + n],
                    Vt_sb[:, 3 * (NJ - 1) + pt: 3 * (NJ - 1) + pt + 1],
                    pend[pt][:, c:c + n],
                    start=False, stop=True)

        # ---- exp over all 67 partitions at once (rows other than
        # 0/32/64 hold exactly 0.0 from the start=True scatter, so they
        # exp to 1.0 and are simply never read downstream) ----
        e9x_sb = const.tile([67, S_local], F32, tag="e9x")
        SCE = []
        _c = 0
        while _c < S_local:
            _n = min(128 if _c < 256 else (256 if _c < 512 else 512),
                     S_local - _c)
            SCE.append((_c, _n))
            _c += _n
        z9_sb = const.tile([67, len(SCE)], F32, tag="z9")
        z3raw = const.tile([3, len(SCE)], F32, tag="z3raw")
        rings = [nc.sync, nc.scalar, nc.gpsimd]
        for ci, (c, n) in enumerate(SCE):
            nc.scalar.activation(e9x_sb[0:67, c:c + n], e9_ps[0:67, c:c + n],
                                 EXP, accum_out=z9_sb[0:67, ci:ci + 1])
        for tt in range(3):
            rings[tt].dma_start(z3raw[tt:tt + 1, :],
                                z9_sb[32 * tt:32 * tt + 1, :])
        nc.scalar.dma_start(z4_d.ap()[:, 0:len(SCE)], z3raw[:])

        ph1.close()  # free u/e PSUM banks for the epilogue pools

        # ---- fused epilogue: transpose exp-scores per s-tile, accumulate
        # numerator N[t, :] = sum_s exp_scores[t, s] * sent[s, :] ----
        trpool = ctx.enter_context(tc.tile_pool(name="tr", bufs=3, space="PSUM"))
        npool = ctx.enter_context(tc.tile_pool(name="n", bufs=2, space="PSUM"))
        eT_sb = const.tile([128, 3 * ST], BF16, tag="eT")
        n_ps = []
        for _hi in range(H2 // 512):
            n_ps_hi = npool.tile([3, 512], F32, tag="n")
            n_ps.append(n_ps_hi)
        def _num_mm(k):
            for hi, hc in enumerate(range(0, H2, 512)):
                nc.tensor.matmul(n_ps[hi][0:3, :],
                                 eT_sb[:, 3 * k:3 * k + 3],
                                 sent_sb[:, k * H2 + hc: k * H2 + hc + 512],
                                 start=(k == 0), stop=(k == ST - 1))
        for k in range(ST):
            tr_ps = trpool.tile([128, 67], F32, tag="tr")
            nc.tensor.transpose(tr_ps[:], e9x_sb[0:67, k * 128:(k + 1) * 128],
                                id67_sb[:])
            # gather head columns {0,32,64}; alternate engines to pipeline
            if k % 2 == 0:
                nc.vector.tensor_copy(eT_sb[:, 3 * k:3 * k + 3],
                                      tr_ps[:, 0:65:32])
            else:
                nc.scalar.copy(eT_sb[:, 3 * k:3 * k + 3], tr_ps[:, 0:65:32])
            if k >= 2:  # numerator MMs two tiles behind: copies never stall PE
                _num_mm(k - 2)
        for k in range(max(0, ST - 2), ST):
            _num_mm(k)
        n_sb = const.tile([3, H2], F32, tag="nsb")
        for hi, hc in enumerate(range(0, H2, 512)):
            if hi % 2 == 0:
                nc.vector.tensor_copy(n_sb[:, hc:hc + 512], n_ps[hi][:])
            else:
                nc.scalar.copy(n_sb[:, hc:hc + 512], n_ps[hi][:])
            nc.sync.dma_start(Ncore_d.ap()[:, hc:hc + 512], n_sb[:, hc:hc + 512])

    nc.compile()
    return nc


def kernel(**inputs):
    global LAST_RESULTS
    from concourse import bass_utils

    sentence = np.ascontiguousarray(
        np.asarray(inputs["sentence"], dtype=np.float32)[0])      # [S, H2]
    length = int(np.asarray(inputs["length"]).reshape(-1)[0])
    if length <= 0:
        return np.zeros((1, H2), dtype=np.float32)
    length = min(length, S)

    ctxs = [inputs["pos_embedding"], inputs["cardinal_phrase_embedding"],
            inputs["headline_embedding"]]
    tags = ["p", "c", "h"]

    # ---- host prep: quantize, fold biases, fp8 error compensation ----
    x8 = (sentence * SX).astype(NP_F8)                            # [S, H2] fp8
    x8f = x8.astype(np.float32) / SX
    dx = sentence - x8f

    bias_all = np.empty((3, A), dtype=np.float32)
    W8_all = np.empty((3, H2, A), dtype=NP_F8)
    v_all = np.empty((3, A), dtype=np.float32)
    corr_all = np.empty((3, S), dtype=np.float32)
    sub = np.arange(0, S, 16)                                     # c_a sample
    for i, tg in enumerate(tags):
        ctx = np.asarray(ctxs[i], dtype=np.float32)[0]            # [E]
        bias = (np.asarray(inputs[f"b_sent_{tg}"], dtype=np.float32)
                + ctx @ np.asarray(inputs[f"W_ctx_{tg}"], dtype=np.float32)
                + np.asarray(inputs[f"b_ctx_{tg}"], dtype=np.float32))
        W = np.asarray(inputs[f"W_sent_{tg}"], dtype=np.float32)
        v = np.asarray(inputs[f"v_{tg}"], dtype=np.float32)
        W8 = (W * SW).astype(NP_F8)
        W8f = W8.astype(np.float32) / SW
        dW = W - W8f
        u_sub = x8f[sub] @ W8f + bias
        c_a = (1.0 - np.tanh(u_sub) ** 2).mean(axis=0)            # [A]
        vc = v * c_a
        corr_all[i] = dx @ (W @ vc) + x8f @ (dW @ vc)
        bias_all[i] = bias
        W8_all[i] = W8
        v_all[i] = v

    S_local = max(128, -(-length // (NCORES * 128)) * 128)        # ceil, 128-aligned
    nc = _cache.get(S_local)
    if nc is None:
        nc = _build(S_local)
        _cache[S_local] = nc

    NJ = A // 128
    KT = H2 // 128
    # Wt8[t, j][p, kt*128 + a] = W8[t, kt*128 + p, j*128 + a]
    Wt8 = np.ascontiguousarray(
        W8_all.reshape(3, KT, 128, NJ, 128)
              .transpose(0, 3, 2, 1, 4).reshape(3, NJ, 128, KT * 128))
    # Vt[p, j*3 + t] = v_t[j*128 + p]
    Vt = np.ascontiguousarray(
        v_all.T.reshape(NJ, 128, 3).transpose(1, 0, 2)
             .reshape(128, NJ * 3)).astype(NP_BF16)
    Bt = np.ascontiguousarray(
        bias_all.T.reshape(NJ, 128, 3).transpose(1, 0, 2).reshape(128, 3 * NJ))
    id67 = np.eye(67, dtype=np.float32)
    sel67 = np.zeros((3, 67), dtype=np.float32)
    for t in range(3):
        sel67[t, 32 * t] = 1.0

    # overflow guard for the shift-free exp: |e| <= ||v||_1 + max|corr|
    e_bound = max(float(np.abs(v_all[t]).sum() + np.abs(corr_all[t]).max())
                  for t in range(3))
    shift = max(0.0, e_bound - 60.0)   # exp arg stays < 60 -> < 1.2e26, Z safe
    if shift:
        corr_all -= shift              # common across cores: cancels in N/Z

    ST = S_local // 128
    SCE_N = 0
    _c = 0
    while _c < S_local:
        _n = min(128 if _c < 256 else (256 if _c < 512 else 512),
                 S_local - _c)
        SCE_N += 1
        _c += _n

    in_maps = []
    for c in range(NCORES):
        s0 = c * S_local
        sl8 = x8[s0:s0 + S_local]
        slf = sentence[s0:s0 + S_local]
        if sl8.shape[0] < S_local:                                 # pad tail core
            pad = S_local - sl8.shape[0]
            sl8 = np.concatenate([sl8, np.zeros((pad, H2), NP_F8)], axis=0)
            slf = np.concatenate([slf, np.zeros((pad, H2), np.float32)], axis=0)
        # chunk-major: sentT8[p, ci*KT*512 + k*n + s'] = x8[s0+ci*512+s', k*128+p]
        slT = sl8.T.reshape(KT, 128, S_local)                      # [k, p, s]
        blocks = [
            np.ascontiguousarray(slT[:, :, cc:cc + nn].transpose(1, 0, 2)
                                 .reshape(128, KT * nn))
            for cc, nn in [(cc0, min(512, S_local - cc0))
                           for cc0 in range(0, S_local, 512)]]
        sentT8 = np.ascontiguousarray(np.concatenate(blocks, axis=1))
        # sentbf[p, k*H2 + h] = sentence[s0 + k*128 + p, h]
        sentbf = np.ascontiguousarray(
            slf.reshape(ST, 128, H2).transpose(1, 0, 2)
               .reshape(128, ST * H2)).astype(NP_BF16)
        smask = np.where((s0 + np.arange(S_local))[None, :] < length,
                         0.0, NEG).astype(np.float32)
        corr3 = np.ascontiguousarray(
            corr_all[:, s0:s0 + S_local] if s0 + S_local <= S else
            np.pad(corr_all[:, s0:S], ((0, 0), (0, s0 + S_local - S))))
        corr3 = (corr3 + smask).astype(np.float32)
        in_maps.append(dict(
            sentT8=sentT8, sentbf=sentbf, Wt8=Wt8, Vt=Vt, Bt=Bt,
            corr3=corr3, sel67=sel67, id67=id67,
        ))

    res = bass_utils.run_bass_kernel_spmd(nc, in_maps,
                                          core_ids=list(range(NCORES)))
    LAST_RESULTS = res

    # ---- exact cross-core combine (shared exp shift cancels in N/Z) ----
    Z = np.stack([res.results[c]["z4"] for c in range(NCORES)])    # [8,3,4]
    Ncore = np.stack([res.results[c]["Ncore"] for c in range(NCORES)])
    Zt = Z[:, :, :SCE_N].astype(np.float64).sum(axis=(0, 2))       # [3]
    Nt = Ncore.astype(np.float64).sum(axis=0)                      # [3,H2]
    out = (Nt / Zt[:, None]).mean(axis=0)
    return out[None, :].astype(np.float32)


# revision 24
# speedup vs baseline: 1.0288x; 1.0135x over previous
"""Trainium2 Bass kernel for triple-head Bahdanau attention (nn_Attention_48258252537865).

v5: like v4 (fp8 DoubleRow u-matmuls + host-side quantization-error
compensation) but the three heads' score matmuls are packed into distinct
PE column groups (tile_position col packing): head t's [K=128, M=1] score
matmul writes PSUM partition 32*t, so the three matmuls execute concurrently
in different 32-column strips of the array instead of serializing three
512-cycle streams on column group 0.  The loop is restructured j-outer /
t-inner so each j's three score matmuls are emitted back-to-back.
"""

import numpy as np
import ml_dtypes
from contextlib import ExitStack

S = 8192
H2 = 1024
A = 2048
NCORES = 8
NEG = -1.0e30
SX = 2.0 ** 5          # sentence pre-scale (fp8)
SW = 2.0 ** 9          # weight pre-scale (fp8)
SCALE_INV = 1.0 / (SX * SW)
NP_F8 = ml_dtypes.float8_e4m3
NP_BF16 = ml_dtypes.bfloat16

_cache = {}
LAST_RESULTS = None  # BassKernelResults of the most recent device run


def _build(S_local):
    import concourse.bacc as bacc
    import concourse.tile as tile
    from concourse import mybir

    F32 = mybir.dt.float32
    F32R = mybir.dt.float32r
    F8 = mybir.dt.float8e4
    BF16 = mybir.dt.bfloat16
    TANH = mybir.ActivationFunctionType.Tanh
    EXP = mybir.ActivationFunctionType.Exp
    DR = mybir.MatmulPerfMode.DoubleRow

    KT = H2 // 128                      # 8 fp8 k-slices of the contraction
    KT2 = KT // 2                       # 4 DoubleRow K=256 tiles
    NJ = A // 128                       # a-tiles per head
    ST = S_local // 128                 # s-tiles (transpose/numerator)
    SC = [(c, min(512, S_local - c)) for c in range(0, S_local, 512)]

    nc = bacc.Bacc("TRN2", target_bir_lowering=False, debug=False,
                   num_devices=NCORES)

    sentT8_d = nc.dram_tensor("sentT8", [128, KT * S_local], F8,
                              kind="ExternalInput")
    sentbf_d = nc.dram_tensor("sentbf", [128, ST * H2], BF16,
                              kind="ExternalInput")
    Wt8_d = nc.dram_tensor("Wt8", [3, NJ, 128, KT * 128], F8,
                           kind="ExternalInput")
    Vt_d = nc.dram_tensor("Vt", [128, NJ * 3], BF16, kind="ExternalInput")
    Bt_d = nc.dram_tensor("Bt", [128, 3 * NJ], F32, kind="ExternalInput")
    corr3_d = nc.dram_tensor("corr3", [3, S_local], F32R, kind="ExternalInput")
    sel67_d = nc.dram_tensor("sel67", [3, 67], F32R, kind="ExternalInput")
    id67_d = nc.dram_tensor("id67", [67, 67], F32, kind="ExternalInput")

    Ncore_d = nc.dram_tensor("Ncore", [3, H2], F32, kind="ExternalOutput")
    z4_d = nc.dram_tensor("z4", [3, 4], F32, kind="ExternalOutput")

    with tile.TileContext(nc) as tc, ExitStack() as ctx:
        const = ctx.enter_context(tc.tile_pool(name="const", bufs=1))
        wpool = ctx.enter_context(tc.tile_pool(name="w", bufs=5))
        thpool = ctx.enter_context(tc.tile_pool(name="th", bufs=7))
        ph1 = ExitStack()
        upool = ph1.enter_context(tc.tile_pool(name="u", bufs=3, space="PSUM"))
        epool = ph1.enter_context(tc.tile_pool(name="e", bufs=1, space="PSUM"))

        # ---- startup (see v4 notes): critical deps first per ring ----
        Wt_sb = {}

        def _wdma(t, j):
            w = wpool.tile([128, KT, 128], F8, tag="w")
            nc.sync.dma_start(
                w[:].rearrange("p k a -> p (k a)"), Wt8_d.ap()[t, j])
            Wt_sb[(t, j)] = w

        sel67_sb = const.tile([3, 67], F32R, tag="sel67")
        id67_sb = const.tile([67, 67], F32, tag="id67")
        Vt_sb = const.tile([128, NJ * 3], BF16, tag="vt")
        Bt_sb = const.tile([128, 3 * NJ], F32, tag="bt")
        corr_sb = const.tile([3, S_local], F32R, tag="corr")
        KH = KT // 2
        w00 = wpool.tile([128, KT, 128], F8, tag="w", name="w00")
        nc.sync.dma_start(w00[:, 0:KH, :].rearrange("p k a -> p (k a)"),
                          Wt8_d.ap()[0, 0][:, 0:KH * 128])
        nc.sync.dma_start(w00[:, KH:KT, :].rearrange("p k a -> p (k a)"),
                          Wt8_d.ap()[0, 0][:, KH * 128:KT * 128])
        Wt_sb[(0, 0)] = w00
        sentT_cs = []
        KQ = KH // 2
        for ci, (c, n) in enumerate(SC):
            sct = const.tile([128, KT, n], F8, tag=f"sentT{ci}", name=f"sT{ci}")
            base = ci * KT * 512
            if ci == 0:
                nc.scalar.dma_start(
                    sct[:, 0:KQ, :].rearrange("p k s -> p (k s)"),
                    sentT8_d.ap()[:, base: base + KQ * n])
                nc.scalar.dma_start(
                    sct[:, KQ:KH, :].rearrange("p k s -> p (k s)"),
                    sentT8_d.ap()[:, base + KQ * n: base + KH * n])
            else:
                nc.scalar.dma_start(
                    sct[:, 0:KH, :].rearrange("p k s -> p (k s)"),
                    sentT8_d.ap()[:, base: base + KH * n])
            sentT_cs.append(sct)
        for ci, (c, n) in enumerate(SC):
            base = ci * KT * 512
            nc.gpsimd.dma_start(
                sentT_cs[ci][:, KH:KT, :].rearrange("p k s -> p (k s)"),
                sentT8_d.ap()[:, base + KH * n: base + KT * n])
        nc.gpsimd.dma_start(Bt_sb[:], Bt_d.ap()[:])
        nc.gpsimd.dma_start(sel67_sb[:], sel67_d.ap()[:])
        nc.gpsimd.dma_start(corr_sb[:], corr3_d.ap()[:])
        nc.gpsimd.dma_start(Vt_sb[:], Vt_d.ap()[:])
        nc.gpsimd.dma_start(id67_sb[:], id67_d.ap()[:])
        _wdma(1, 0)
        _wdma(2, 0)
        _wdma(0, 1)

        sent_sb = const.tile([128, ST * H2], BF16, tag="sent")

        # ---- score accumulator: head t's row on PSUM partition 32t ----
        e9_ps = epool.tile([67, S_local], F32, tag="e")

        # ---- u (fp8 DoubleRow) -> tanh -> col-packed score triples ----
        pend = None  # [th0, th1, th2] of the previous j
        for j in range(NJ):
            ths = []
            for t in range(3):
                wtile = Wt_sb.pop((t, j), None)
                if wtile is None:
                    wtile = wpool.tile([128, KT, 128], F8, tag="w")
                    nc.sync.dma_start(
                        wtile[:].rearrange("p k a -> p (k a)"), Wt8_d.ap()[t, j])
                u_ps = upool.tile([128, S_local], F32, tag="u")
                if j == 0 and t == 0:
                    for ci, (c, n) in enumerate(SC):
                        for kt in range(KT2):
                            nc.tensor.matmul(
                                u_ps[:, c:c + n],
                                wtile[:, 2 * kt:2 * kt + 2, :],
                                sentT_cs[ci][:, 2 * kt:2 * kt + 2, :],
                                start=(kt == 0), stop=(kt == KT2 - 1),
                                perf_mode=DR)
                else:
                    for kt in range(KT2):
                        for ci, (c, n) in enumerate(SC):
                            nc.tensor.matmul(
                                u_ps[:, c:c + n],
                                wtile[:, 2 * kt:2 * kt + 2, :],
                                sentT_cs[ci][:, 2 * kt:2 * kt + 2, :],
                                start=(kt == 0), stop=(kt == KT2 - 1),
                                perf_mode=DR)
                if t == 1 and pend is not None:
                    # score triples of j-1: per chunk, the 3 heads go to
                    # column groups 0/32/64 and run concurrently
                    for (c, n) in SC:
                        for pt in range(3):
                            nc.tensor.matmul(
                                e9_ps[32 * pt:32 * pt + 1, c:c + n],
                                Vt_sb[:, 3 * (j - 1) + pt: 3 * (j - 1) + pt + 1],
                                pend[pt][:, c:c + n],
                                start=False, stop=False)
                th = thpool.tile([128, S_local], BF16, tag="th",
                                 name=f"th{t}_{j}")
                if j == NJ - 1:
                    # final j: per-chunk tanh so the last score triples can
                    # start as soon as their chunk is ready
                    for (c, n) in SC:
                        nc.scalar.activation(
                            th[:, c:c + n], u_ps[:, c:c + n], TANH,
                            bias=Bt_sb[:, j * 3 + t: j * 3 + t + 1],
                            scale=SCALE_INV)
                else:
                    nc.scalar.activation(th[:], u_ps[:], TANH,
                                         bias=Bt_sb[:, j * 3 + t: j * 3 + t + 1],
                                         scale=SCALE_INV)
                ths.append(th)
                if 1 <= j <= 3 and (j - 1) * 3 + t < ST:
                    # trickle the 2MB numerator operand behind the W stream
                    pc = (j - 1) * 3 + t
                    nc.sync.dma_start(sent_sb[:, pc * H2:(pc + 1) * H2],
                                      sentbf_d.ap()[:, pc * H2:(pc + 1) * H2])
                if j == 0 and t == 0:
                    # key mask + fp8 error compensation: one K=3 matmul with
                    # a [3, 67] selection stationary scatters head t's corr
                    # row onto partition 32t and zero-starts the accumulator
                    for (c, n) in SC:
                        nc.tensor.matmul(
                            e9_ps[0:67, c:c + n], sel67_sb[:],
                            corr_sb[0:3, c:c + n],
                            start=True, stop=False)
            pend = ths
        for (c, n) in SC:
            for pt in range(3):
                nc.tensor.matmul(
                    e9_ps[32 * pt:32 * pt + 1, c:c + n],
                    Vt_sb[:, 3 * (NJ - 1) + pt: 3 * (NJ - 1) + pt + 1],
                    pend[pt][:, c:c + n],
                    start=False, stop=True)

        # ---- exp over all 67 partitions at once (rows other than
        # 0/32/64 hold exactly 0.0 from the start=True scatter, so they
        # exp to 1.0 and are simply never read downstream) ----
        e9x_sb = const.tile([67, S_local], F32, tag="e9x")
        SCE = []
        _c = 0
        while _c < S_local:
            _n = min(128 if _c < 256 else (256 if _c < 512 else 512),
                     S_local - _c)
            SCE.append((_c, _n))
            _c += _n
        z9_sb = const.tile([67, len(SCE)], F32, tag="z9")
        z3raw = const.tile([3, len(SCE)], F32, tag="z3raw")
        rings = [nc.sync, nc.scalar, nc.gpsimd]
        for ci, (c, n) in enumerate(SCE):
            nc.scalar.activation(e9x_sb[0:67, c:c + n], e9_ps[0:67, c:c + n],
                                 EXP, accum_out=z9_sb[0:67, ci:ci + 1])
        for tt in range(3):
            rings[tt].dma_start(z3raw[tt:tt + 1, :],
                                z9_sb[32 * tt:32 * tt + 1, :])
        nc.scalar.dma_start(z4_d.ap()[:, 0:len(SCE)], z3raw[:])

        ph1.close()  # free u/e PSUM banks for the epilogue pools

        # ---- fused epilogue: transpose exp-scores per s-tile, accumulate
        # numerator N[t, :] = sum_s exp_scores[t, s] * sent[s, :] ----
        trpool = ctx.enter_context(tc.tile_pool(name="tr", bufs=3, space="PSUM"))
        npool = ctx.enter_context(tc.tile_pool(name="n", bufs=1, space="PSUM"))
        eT_sb = const.tile([128, 3 * ST], BF16, tag="eT")
        # both H2 halves in one [35, 512] PSUM tile: half hi lands on
        # partitions 32*hi..32*hi+2 so the two matmuls per s-tile run in
        # different PE column groups concurrently (same pattern as the
        # score triples)
        n2_ps = npool.tile([35, 512], F32, tag="n")
        def _num_mm(k):
            for hi, hc in enumerate(range(0, H2, 512)):
                nc.tensor.matmul(n2_ps[32 * hi:32 * hi + 3, :],
                                 eT_sb[:, 3 * k:3 * k + 3],
                                 sent_sb[:, k * H2 + hc: k * H2 + hc + 512],
                                 start=(k == 0), stop=(k == ST - 1))
        for k in range(ST):
            tr_ps = trpool.tile([128, 67], F32, tag="tr")
            nc.tensor.transpose(tr_ps[:], e9x_sb[0:67, k * 128:(k + 1) * 128],
                                id67_sb[:])
            # gather head columns {0,32,64}; alternate engines to pipeline
            if k % 2 == 0:
                nc.vector.tensor_copy(eT_sb[:, 3 * k:3 * k + 3],
                                      tr_ps[:, 0:65:32])
            else:
                nc.scalar.copy(eT_sb[:, 3 * k:3 * k + 3], tr_ps[:, 0:65:32])
            if k >= 2:  # numerator MMs two tiles behind: copies never stall PE
                _num_mm(k - 2)
        for k in range(max(0, ST - 2), ST):
            _num_mm(k)
        n_sb = const.tile([35, 512], F32, tag="nsb")
        nc.vector.tensor_copy(n_sb[0:3, :], n2_ps[0:3, :])
        nc.sync.dma_start(Ncore_d.ap()[:, 0:512], n_sb[0:3, :])
        nc.scalar.copy(n_sb[32:35, :], n2_ps[32:35, :])
        nc.scalar.dma_start(Ncore_d.ap()[:, 512:1024], n_sb[32:35, :])

    nc.compile()
    return nc


def kernel(**inputs):
    global LAST_RESULTS
    from concourse import bass_utils

    sentence = np.ascontiguousarray(
        np.asarray(inputs["sentence"], dtype=np.float32)[0])      # [S, H2]
    length = int(np.asarray(inputs["length"]).reshape(-1)[0])
    if length <= 0:
        return np.zeros((1, H2), dtype=np.float32)
    length = min(length, S)

    ctxs = [inputs["pos_embedding"], inputs["cardinal_phrase_embedding"],
            inputs["headline_embedding"]]
    tags = ["p", "c", "h"]

    # ---- host prep: quantize, fold biases, fp8 error compensation ----
    x8 = (sentence * SX).astype(NP_F8)                            # [S, H2] fp8
    x8f = x8.astype(np.float32) / SX
    dx = sentence - x8f

    bias_all = np.empty((3, A), dtype=np.float32)
    W8_all = np.empty((3, H2, A), dtype=NP_F8)
    v_all = np.empty((3, A), dtype=np.float32)
    corr_all = np.empty((3, S), dtype=np.float32)
    sub = np.arange(0, S, 16)                                     # c_a sample
    for i, tg in enumerate(tags):
        ctx = np.asarray(ctxs[i], dtype=np.float32)[0]            # [E]
        bias = (np.asarray(inputs[f"b_sent_{tg}"], dtype=np.float32)
                + ctx @ np.asarray(inputs[f"W_ctx_{tg}"], dtype=np.float32)
                + np.asarray(inputs[f"b_ctx_{tg}"], dtype=np.float32))
        W = np.asarray(inputs[f"W_sent_{tg}"], dtype=np.float32)
        v = np.asarray(inputs[f"v_{tg}"], dtype=np.float32)
        W8 = (W * SW).astype(NP_F8)
        W8f = W8.astype(np.float32) / SW
        dW = W - W8f
        u_sub = x8f[sub] @ W8f + bias
        c_a = (1.0 - np.tanh(u_sub) ** 2).mean(axis=0)            # [A]
        vc = v * c_a
        corr_all[i] = dx @ (W @ vc) + x8f @ (dW @ vc)
        bias_all[i] = bias
        W8_all[i] = W8
        v_all[i] = v

    S_local = max(128, -(-length // (NCORES * 128)) * 128)        # ceil, 128-aligned
    nc = _cache.get(S_local)
    if nc is None:
        nc = _build(S_local)
        _cache[S_local] = nc

    NJ = A // 128
    KT = H2 // 128
    # Wt8[t, j][p, kt*128 + a] = W8[t, kt*128 + p, j*128 + a]
    Wt8 = np.ascontiguousarray(
        W8_all.reshape(3, KT, 128, NJ, 128)
              .transpose(0, 3, 2, 1, 4).reshape(3, NJ, 128, KT * 128))
    # Vt[p, j*3 + t] = v_t[j*128 + p]
    Vt = np.ascontiguousarray(
        v_all.T.reshape(NJ, 128, 3).transpose(1, 0, 2)
             .reshape(128, NJ * 3)).astype(NP_BF16)
    Bt = np.ascontiguousarray(
        bias_all.T.reshape(NJ, 128, 3).transpose(1, 0, 2).reshape(128, 3 * NJ))
    id67 = np.eye(67, dtype=np.float32)
    sel67 = np.zeros((3, 67), dtype=np.float32)
    for t in range(3):
        sel67[t, 32 * t] = 1.0

    # overflow guard for the shift-free exp: |e| <= ||v||_1 + max|corr|
    e_bound = max(float(np.abs(v_all[t]).sum() + np.abs(corr_all[t]).max())
                  for t in range(3))
    shift = max(0.0, e_bound - 60.0)   # exp arg stays < 60 -> < 1.2e26, Z safe
    if shift:
        corr_all -= shift              # common across cores: cancels in N/Z

    ST = S_local // 128
    SCE_N = 0
    _c = 0
    while _c < S_local:
        _n = min(128 if _c < 256 else (256 if _c < 512 else 512),
                 S_local - _c)
        SCE_N += 1
        _c += _n

    in_maps = []
    for c in range(NCORES):
        s0 = c * S_local
        sl8 = x8[s0:s0 + S_local]
        slf = sentence[s0:s0 + S_local]
        if sl8.shape[0] < S_local:                                 # pad tail core
            pad = S_local - sl8.shape[0]
            sl8 = np.concatenate([sl8, np.zeros((pad, H2), NP_F8)], axis=0)
            slf = np.concatenate([slf, np.zeros((pad, H2), np.float32)], axis=0)
        # chunk-major: sentT8[p, ci*KT*512 + k*n + s'] = x8[s0+ci*512+s', k*128+p]
        slT = sl8.T.reshape(KT, 128, S_local)                      # [k, p, s]
        blocks = [
            np.ascontiguousarray(slT[:, :, cc:cc + nn].transpose(1, 0, 2)
                                 .reshape(128, KT * nn))
            for cc, nn in [(cc0, min(512, S_local - cc0))
                           for cc0 in range(0, S_local, 512)]]
        sentT8 = np.ascontiguousarray(np.concatenate(blocks, axis=1))
        # sentbf[p, k*H2 + h] = sentence[s0 + k*128 + p, h]
        sentbf = np.ascontiguousarray(
            slf.reshape(ST, 128, H2).transpose(1, 0, 2)
               .reshape(128, ST * H2)).astype(NP_BF16)
        smask = np.where((s0 + np.arange(S_local))[None, :] < length,
                         0.0, NEG).astype(np.float32)
        corr3 = np.ascontiguousarray(
            corr_all[:, s0:s0 + S_local] if s0 + S_local <= S else
            np.pad(corr_all[:, s0:S], ((0, 0), (0, s0 + S_local - S))))
        corr3 = (corr3 + smask).astype(np.float32)
        in_maps.append(dict(
            sentT8=sentT8, sentbf=sentbf, Wt8=Wt8, Vt=Vt, Bt=Bt,
            corr3=corr3, sel67=sel67, id67=id67,
        ))

    res = bass_utils.run_bass_kernel_spmd(nc, in_maps,
                                          core_ids=list(range(NCORES)))
    LAST_RESULTS = res

    # ---- exact cross-core combine (shared exp shift cancels in N/Z) ----
    Z = np.stack([res.results[c]["z4"] for c in range(NCORES)])    # [8,3,4]
    Ncore = np.stack([res.results[c]["Ncore"] for c in range(NCORES)])
    Zt = Z[:, :, :SCE_N].astype(np.float64).sum(axis=(0, 2))       # [3]
    Nt = Ncore.astype(np.float64).sum(axis=0)                      # [3,H2]
    out = (Nt / Zt[:, None]).mean(axis=0)
    return out[None, :].astype(np.float32)
